# revision 26
# baseline (speedup 1.0000x reference)
"""Multi-head attention (B=4, L=2048, E=1024, H=16, DK=64) on 8 TRN2 cores.

Sharding: core c -> (batch b = c//2, head-group g = c%2 of 8 heads).

Structure (v2): one-head attention blocks with a software-pipelined
S -> exp -> AV loop (st double-buffered so exp(kk) overlaps S(kk+1)),
and the qkv projection + fc matmuls interleaved into the attention
blocks' spare PE slots. This keeps the PE near-continuously busy --
important because the PE HAM clock-gate halves the PE clock when it
sees idle windows -- and hides the qkv/fc phases entirely under the
ACT-bound attention loop.

PSUM budget (8 banks): st [128,1024]x2 bufs (4) + av [128,1024] (2)
+ background qkv/fc tile [128,1024] (2).

Per core: qkv projection for its batch+heads (bf16 matmuls, fp32
accum), attention for 8 (b,h) pairs, partial fc over its 512 features,
then pair-wise ReduceScatters so each core emits a disjoint slice of
the final output. Host assembles the full [4, 2048, 1024] output.

Self-contained: hardcodes all shapes; requires only the concourse stack.
"""

import numpy as np
import ml_dtypes

try:
    import axon_prof

    axon_prof.install()
except Exception:
    pass

import concourse.mybir as mybir
import concourse.tile as tile
from concourse import bacc
from concourse import bass_utils

B, L, E = 4, 2048, 1024
H, DK = 16, 64
H8 = 8                      # heads per core
F = H8 * 3 * DK             # qkv features per core = 1536
FO = H8 * DK                # attn-out features per core = 512
NCORES = 8
LHALF = L // 2

f32 = mybir.dt.float32
bf16 = mybir.dt.bfloat16
Exp = mybir.ActivationFunctionType.Exp
MUL = mybir.AluOpType.mult
ADD = mybir.AluOpType.add

_CACHE = {}


def build_nc():
    nc = bacc.Bacc("TRN2", target_bir_lowering=False, debug=False, num_devices=NCORES)

    # x arrives already transposed (host-side) so no xbar transpose is needed
    x = nc.dram_tensor("x", [E, L], bf16, kind="ExternalInput")
    w_qkv = nc.dram_tensor("w_qkv", [E, F], bf16, kind="ExternalInput")
    b_qkv = nc.dram_tensor("b_qkv", [128, 12], f32, kind="ExternalInput")
    w_fc = nc.dram_tensor("w_fc", [FO, E], bf16, kind="ExternalInput")
    b_fc = nc.dram_tensor("b_fc", [1, E], f32, kind="ExternalInput")
    # bf16 output: the ReduceScatters write their scattered slices directly
    # here (no SWDGE cast pass); the host upcasts to f32
    out = nc.dram_tensor("out", [LHALF, E], bf16, kind="ExternalOutput")

    with tile.TileContext(nc) as tc:
        with (
            tc.tile_pool(name="persist", bufs=1) as pp,
            tc.tile_pool(name="work", bufs=3) as wp,
            tc.tile_pool(name="stage", bufs=1) as sp,
            tc.tile_pool(name="srsp", bufs=2) as srs_pool,
            tc.tile_pool(name="ys", bufs=3) as yp_pool,
            tc.tile_pool(name="dram", bufs=1, space="DRAM") as dram,
        ):
            # ---- persistent SBUF ----
            xT = pp.tile([128, 8, L], bf16, tag="xT")          # X^T  4 MiB
            wq = pp.tile([128, 8, F], bf16, tag="wq")          # 3 MiB
            bq = pp.tile([128, 12], f32, tag="bq")
            wfc = pp.tile([128, 4, E], bf16, tag="wfc")        # 1 MiB
            bias = pp.tile([128, E], f32, tag="bias")          # 0.5 MiB
            qt = pp.tile([128, 4, L], bf16, tag="qt")          # Q^T 2 MiB
            kt = pp.tile([128, 4, L], bf16, tag="kt")          # K^T 2 MiB
            vt = sp.tile([128, 4, L], bf16, tag="vt")          # V^T staging 2 MiB
            # V natural layout, 80-elem stride; col 64 holds the ones column
            # so AV matmuls with lhsT [V|1] (M=65) produce rowsums for free
            v = pp.tile([128, H8, 16, 80], bf16, tag="v")      # 2.5 MiB
            onT = pp.tile([128, 4, L], bf16, tag="onT")        # attn out^T 2 MiB

            # ---- input DMAs (plain copies, split across both HWDGE queues)
            for e in range(8):
                nc.scalar.dma_start(
                    wq[:, e, :], w_qkv[e * 128 : (e + 1) * 128, :]
                )
                nc.sync.dma_start(xT[:, e, :], x[e * 128 : (e + 1) * 128, :])
                if e == 0:
                    nc.scalar.dma_start(bq[:], b_qkv[:])
            nc.sync.dma_start(wfc[:], w_fc.rearrange("(c p) e -> p c e", p=128))
            bfc_row = pp.tile([1, E], f32, tag="bfc_row")
            nc.sync.dma_start(bfc_row[:], b_fc[:])
            nc.gpsimd.partition_broadcast(bias[:], bfc_row[:])
            nc.vector.memset(v[:, :, :, 64:65], 1.0)

            # exp table preload: the ~2.7us PSEUDO_LOAD_ACT_FUNC_SET runs
            # under the input DMAs instead of at the first real exp
            dummy = pp.tile([1, 8], f32, tag="dummy")
            nc.vector.memset(dummy[:], 0.0)
            nc.scalar.activation(dummy[:], dummy[:], Exp)

            # one DRAM tile per RS chunk: a sliced shared tile would give the
            # collectives tile-coarse false deps against later fc writes,
            # which stalls the sync queue (and transitively the DVE chain)
            rs_in = [
                dram.tile([512, E], bf16, name=f"rs_in{i}", tag=f"rs_in{i}")
                for i in range(4)
            ]
            PAIRS = [[0, 1], [2, 3], [4, 5], [6, 7]]
            rs_outc = [
                dram.tile([256, E], bf16, name=f"rs_outc{i}", tag=f"rs_outc{i}")
                for i in range(4)
            ]

            def rs_chunk(i):
                # ReduceScatter of 512 fc-output tokens (chunk i of 4):
                # core g of a pair gets tokens i*512 + g*256 + [0:256]
                nc.gpsimd.collective_compute(
                    "ReduceScatter", ADD, replica_groups=PAIRS,
                    ins=[rs_in[i].opt()], outs=[rs_outc[i].opt()],
                )

            with (
                tc.tile_pool(name="pst", bufs=2, space="PSUM") as pst_pool,
                tc.tile_pool(name="psav", bufs=1, space="PSUM") as psav_pool,
                tc.tile_pool(name="psbg", bufs=1, space="PSUM") as psbg_pool,
            ):
                # ---- HAM warm-up: ~40 junk matmuls into the (not yet
                # used) av slot while the input DMAs land, so the PE clock
                # gate is already at full rate when real work starts.
                warm = psav_pool.tile([128, LHALF], f32, tag="av", name="av")
                for _ in range(40):
                    nc.tensor.matmul(
                        warm[0:64, 0:512], wq[:, 0, 0:64], wq[:, 0, 0:512],
                        start=True, stop=True,
                    )

                # ---- background PE work: qkv projection groups and fc
                # chunks, emitted a few matmuls at a time from inside the
                # attention loop (fills PE slack, keeps the HAM clock warm).
                # Each unit is a closure emitting ~1 FD-512 matmul.
                bg_units = []

                def qkv_group(ft, tb):
                    """ft-tile [128f, 512tok]: 8 matmuls + bias eviction
                    into qt/kt/vt. bg tile is [128,512] double-buffered so
                    the eviction overlaps the next group's matmuls."""
                    bgt = psbg_pool.tile([128, 512], f32, tag="bg", name="bg")
                    for kc in range(8):
                        lhsT = wq[:, kc, ft * 128 : (ft + 1) * 128]
                        yield lambda kc=kc, lhsT=lhsT, bgt=bgt: (
                            nc.tensor.matmul(
                                bgt[:],
                                lhsT,
                                xT[:, kc, tb * 512 : (tb + 1) * 512],
                                start=(kc == 0),
                                stop=(kc == 7),
                            )
                        )
                    if ft < 4:
                        dst = qt[:, ft, tb * 512 : (tb + 1) * 512]
                    elif ft < 8:
                        dst = kt[:, ft - 4, tb * 512 : (tb + 1) * 512]
                    else:
                        dst = vt[:, ft - 8, tb * 512 : (tb + 1) * 512]

                    def evict(bgt=bgt, dst=dst, ft=ft, tb=tb):
                        nc.vector.tensor_scalar_add(dst, bgt[:], bq[:, ft : ft + 1])
                        if ft >= 8 and tb == 3:
                            # V^T -> V (token-major) via xbar transpose
                            for h in (2 * (ft - 8), 2 * (ft - 8) + 1):
                                nc.sync.dma_start_transpose(
                                    v[:, h, :, 0:DK],
                                    vt[(h % 2) * 64 : (h % 2) * 64 + 64, h // 2, :],
                                )

                    yield evict

                def fc_chunk(qb, t8):
                    """fc for one 128-token chunk: two 4-matmul column
                    halves on the [128,512] bg tile + bias/cast/DMA."""
                    t = qb * 8 + t8
                    ys = yp_pool.tile([128, E], bf16, tag="ys")
                    for e2 in range(2):
                        yp = psbg_pool.tile([128, 512], f32, tag="bg", name="bg")
                        for c in range(4):
                            lhsT = onT[:, c, t * 128 : (t + 1) * 128]
                            yield lambda c=c, e2=e2, lhsT=lhsT, yp=yp: (
                                nc.tensor.matmul(
                                    yp[:],
                                    lhsT,
                                    wfc[:, c, e2 * 512 : (e2 + 1) * 512],
                                    start=(c == 0),
                                    stop=(c == 3),
                                )
                            )

                        def evict_half(yp=yp, e2=e2):
                            nc.vector.tensor_tensor(
                                ys[:, e2 * 512 : (e2 + 1) * 512], yp[:],
                                bias[:, e2 * 512 : (e2 + 1) * 512], op=ADD,
                            )

                        yield evict_half

                    def send(ys=ys, qb=qb, t8=t8):
                        nc.sync.dma_start(
                            rs_in[2 * qb + t8 // 4][(t8 % 4) * 128 : (t8 % 4) * 128 + 128, :],
                            ys[:],
                        )

                    yield send

                def bg_push(gen):
                    # keep generators lazy: a group's PSUM tile is allocated
                    # when its first unit is pulled, so slot-cycling order
                    # matches emission order
                    bg_units.append(gen)

                def bg_run(n):
                    done = 0
                    while done < n and bg_units:
                        u = next(bg_units[0], None)
                        if u is None:
                            bg_units.pop(0)
                            continue
                        u()
                        done += 1

                def bg_run_all():
                    while bg_units:
                        u = next(bg_units[0], None)
                        if u is None:
                            bg_units.pop(0)
                            continue
                        u()

                # qkv emission order, per head-pair: V (all tokens, feeds
                # the transposes), Q for the qb=0 half only, K (all
                # tokens). Q's qb=1 halves are deferred to the end -- the
                # qb=1 blocks only need them much later.
                QKV_GROUPS = []
                for jp in range(4):
                    QKV_GROUPS += [(8 + jp, tb) for tb in range(4)]
                    QKV_GROUPS += [(jp, 0), (jp, 1)]
                    QKV_GROUPS += [(4 + jp, tb) for tb in range(4)]
                QKV_GROUPS += [(jp, tb) for jp in range(4) for tb in (2, 3)]
                for ft, tb in QKV_GROUPS:
                    bg_push(qkv_group(ft, tb))

                # ---- pre-attention: exactly the groups blocks 0-1 read
                # (V ft8 all tb, Q ft0 tb0-1, K ft4 all tb = 10 groups).
                # Consumers must not be emitted before their producers --
                # the tile dep tracker only orders backward in emission.
                bg_run(10 * 9)

                def attn_block(qb, h, bg_per_kk=2):
                    """One head, one q-half (1024 tokens): 16-kk pipelined
                    S -> exp -> AV with st double-buffered."""
                    j, p0 = h // 2, 64 * (h % 2)
                    av = psav_pool.tile([128, LHALF], f32, tag="av", name="av")
                    pts = {}

                    def emit_st(kk):
                        st = pst_pool.tile([128, LHALF], f32, tag="st", name="st")
                        for u in range(2):
                            q0 = qb * LHALF + u * 512
                            nc.tensor.matmul(
                                st[:, u * 512 : (u + 1) * 512],
                                kt[p0 : p0 + 64, j, kk * 128 : (kk + 1) * 128],
                                qt[p0 : p0 + 64, j, q0 : q0 + 512],
                                start=True,
                                stop=True,
                            )
                        pt = wp.tile([128, LHALF], bf16, tag="pt")
                        nc.scalar.activation(pt[:], st[:], Exp, scale=0.125)
                        pts[kk] = pt

                    def emit_av(kk):
                        pt = pts.pop(kk)
                        first, last = kk == 0, kk == 15
                        for u in range(2):
                            sl = slice(u * 512, (u + 1) * 512)
                            nc.tensor.matmul(
                                av[0:65, sl], v[:, h, kk, 0:65], pt[:, sl],
                                start=first, stop=last,
                            )

                    for kk in range(16):
                        emit_st(kk)
                        bg_run(bg_per_kk)
                        if kk > 0:
                            emit_av(kk - 1)
                    emit_av(15)

                    # early evict (frees av psum): unnormalized out^T.
                    # odd heads' rows must land on partitions 64:128 -> DMA
                    # shift through a bf16 staging tile.
                    qsl = slice(qb * LHALF, (qb + 1) * LHALF)
                    srs = srs_pool.tile([128, 2 * LHALF], f32, tag="srs")
                    if p0 == 0:
                        nc.vector.tensor_copy(onT[0:64, j, qsl], av[0:64, :])
                    else:
                        tmp = wp.tile([64, LHALF], bf16, tag="tmp")
                        nc.vector.tensor_copy(tmp[:], av[0:64, :])
                        nc.sync.dma_start(onT[64:128, j, qsl], tmp[:])
                    nc.vector.tensor_copy(srs[64:65, 0:LHALF], av[64:65, :])
                    # deferred normalization (overlaps the next block):
                    # srs cols 0:1024 = sums row, 1024:2048 = broadcast
                    nc.sync.dma_start(srs[0:1, 0:LHALF], srs[64:65, 0:LHALF])
                    nc.gpsimd.partition_broadcast(
                        srs[:, LHALF : 2 * LHALF], srs[0:1, 0:LHALF]
                    )
                    nc.vector.reciprocal_approx_fast(
                        srs[:, LHALF : 2 * LHALF], srs[:, LHALF : 2 * LHALF]
                    )
                    nc.vector.tensor_tensor(
                        onT[p0 : p0 + 64, j, qsl], onT[p0 : p0 + 64, j, qsl],
                        srs[p0 : p0 + 64, LHALF : 2 * LHALF], op=MUL,
                    )

                # qb=0 half: 8 one-head blocks; remaining qkv groups drain
                # into their PE slack (front-loaded so ft deadlines for
                # later heads are met without stalls).
                for h, rate in zip(range(8), (4, 4, 4, 3, 3, 3, 3, 3)):
                    attn_block(0, h, rate)
                bg_run_all()
                # fc for qb=0 spreads over the early qb=1 blocks; each RS
                # chunk fires as soon as its 4 fc chunks are in DRAM, so
                # the collectives overlap the remaining attention.
                for t8 in range(8):
                    bg_push(fc_chunk(0, t8))
                attn_block(1, 0, 2)
                attn_block(1, 1, 2)
                rs_chunk(0)
                attn_block(1, 2, 2)
                attn_block(1, 3, 2)
                bg_run_all()
                rs_chunk(1)
                for h in range(4, 8):
                    attn_block(1, h, 1)
                # fc for qb=1: the only serial tail, with its RS split in
                # two so RS#2a overlaps the second half of fc
                for t8 in range(4):
                    bg_push(fc_chunk(1, t8))
                bg_run_all()
                rs_chunk(2)
                for t8 in range(4, 8):
                    bg_push(fc_chunk(1, t8))
                bg_run_all()
                rs_chunk(3)
                # output copies last: the first three RS chunks are long
                # done, so only the copy of chunk 3 waits on anything
                for i in range(4):
                    nc.gpsimd.dma_start(
                        out[i * 256 : (i + 1) * 256, :], rs_outc[i][:]
                    )

    nc.finalize()
    return nc


def _prep_inputs(X, W_qkv, b_qkv, W_fc, b_fc):
    """Host-side shard + permute + cast. Returns in_maps for 8 cores."""
    X = np.asarray(X, dtype=np.float32)
    W_qkv = np.asarray(W_qkv, dtype=np.float32)
    b_qkv = np.asarray(b_qkv, dtype=np.float32)
    W_fc = np.asarray(W_fc, dtype=np.float32)
    b_fc = np.asarray(b_fc, dtype=np.float32)

    in_maps = []
    bfc_half = (0.5 * b_fc).astype(np.float32).reshape(1, E)
    for c in range(NCORES):
        b, g = divmod(c, 2)
        heads = np.arange(g * H8, (g + 1) * H8)
        # column order: all Q feats (head-major), then K, then V
        cols = np.concatenate(
            [
                np.concatenate([h * 3 * DK + off + np.arange(DK) for h in heads])
                for off in (0, DK, 2 * DK)
            ]
        )
        wq_sh = W_qkv[:, cols].astype(ml_dtypes.bfloat16)
        bq_sh = b_qkv[cols].astype(np.float32).reshape(12, 128).T.copy()
        wfc_sh = W_fc[g * FO : (g + 1) * FO, :].astype(ml_dtypes.bfloat16)
        in_maps.append(
            {
                "x": np.ascontiguousarray(X[b].T).astype(ml_dtypes.bfloat16),
                "w_qkv": wq_sh,
                "b_qkv": np.ascontiguousarray(bq_sh),
                "w_fc": wfc_sh,
                "b_fc": bfc_half,
            }
        )
    return in_maps


def run_kernel(inputs, trace=False):
    if "nc" not in _CACHE:
        _CACHE["nc"] = build_nc()
    nc = _CACHE["nc"]
    in_maps = _prep_inputs(**inputs)
    res = bass_utils.run_bass_kernel_spmd(
        nc, in_maps, core_ids=list(range(NCORES)), trace=trace
    )
    Y = np.empty((B, L, E), dtype=np.float32)
    Q4 = LHALF // 4
    for c in range(NCORES):
        b, g = divmod(c, 2)
        o = np.asarray(res.results[c]["out"], dtype=np.float32)
        # RS chunk i scattered tokens [i*512:(i+1)*512]; core g of a pair
        # holds tokens i*512 + g*256 + [0:256] in out rows [i*256:(i+1)*256]
        for i in range(4):
            Y[b, i * 512 + g * Q4 : i * 512 + (g + 1) * Q4, :] = (
                o[i * Q4 : (i + 1) * Q4]
            )
    return Y, res


def kernel(X, W_qkv, b_qkv, W_fc, b_fc):
    Y, _ = run_kernel(
        dict(X=X, W_qkv=W_qkv, b_qkv=b_qkv, W_fc=W_fc, b_fc=b_fc), trace=False
    )
    return Y


# revision 46
# speedup vs baseline: 1.0717x; 1.0717x over previous
"""Multi-head attention (B=4, L=2048, E=1024, H=16, DK=64) on 8 TRN2 cores.

Sharding: core c -> (batch b = c//2, head-group g = c%2 of 8 heads).

Structure (v2): one-head attention blocks with a software-pipelined
S -> exp -> AV loop (st double-buffered so exp(kk) overlaps S(kk+1)),
and the qkv projection + fc matmuls interleaved into the attention
blocks' spare PE slots. This keeps the PE near-continuously busy --
important because the PE HAM clock-gate halves the PE clock when it
sees idle windows -- and hides the qkv/fc phases entirely under the
ACT-bound attention loop.

PSUM budget (8 banks): st [128,1024]x2 bufs (4) + av [128,1024] (2)
+ background qkv/fc tile [128,1024] (2).

Per core: qkv projection for its batch+heads (bf16 matmuls, fp32
accum), attention for 8 (b,h) pairs, partial fc over its 512 features,
then pair-wise ReduceScatters so each core emits a disjoint slice of
the final output. Host assembles the full [4, 2048, 1024] output.

Self-contained: hardcodes all shapes; requires only the concourse stack.
"""

import numpy as np
import ml_dtypes

try:
    import axon_prof

    axon_prof.install()
except Exception:
    pass

import concourse.mybir as mybir
import concourse.tile as tile
from concourse import bacc
from concourse import bass_utils

B, L, E = 4, 2048, 1024
H, DK = 16, 64
H8 = 8                      # heads per core
F = H8 * 3 * DK             # qkv features per core = 1536
FO = H8 * DK                # attn-out features per core = 512
NCORES = 8
LHALF = L // 2

f32 = mybir.dt.float32
bf16 = mybir.dt.bfloat16
Exp = mybir.ActivationFunctionType.Exp
MUL = mybir.AluOpType.mult
ADD = mybir.AluOpType.add

_CACHE = {}


def build_nc():
    nc = bacc.Bacc("TRN2", target_bir_lowering=False, debug=False, num_devices=NCORES)

    # x arrives already transposed (host-side) so no xbar transpose is needed
    x = nc.dram_tensor("x", [E, L], bf16, kind="ExternalInput")
    w_qkv = nc.dram_tensor("w_qkv", [E, F], bf16, kind="ExternalInput")
    b_qkv = nc.dram_tensor("b_qkv", [128, 12], f32, kind="ExternalInput")
    w_fc = nc.dram_tensor("w_fc", [FO, E], bf16, kind="ExternalInput")
    b_fc = nc.dram_tensor("b_fc", [1, E], f32, kind="ExternalInput")
    # bf16 output: the ReduceScatters write their scattered slices directly
    # here (no SWDGE cast pass); the host upcasts to f32
    out = nc.dram_tensor("out", [LHALF, E], bf16, kind="ExternalOutput")

    with tile.TileContext(nc) as tc:
        with (
            tc.tile_pool(name="persist", bufs=1) as pp,
            tc.tile_pool(name="work", bufs=3) as wp,
            tc.tile_pool(name="stage", bufs=1) as sp,
            tc.tile_pool(name="rrp", bufs=2) as srs_pool,
            tc.tile_pool(name="ys", bufs=3) as yp_pool,
            tc.tile_pool(name="dram", bufs=1, space="DRAM") as dram,
        ):
            # ---- persistent SBUF ----
            xT = pp.tile([128, 8, L], bf16, tag="xT")          # X^T  4 MiB
            wq = pp.tile([128, 8, F], bf16, tag="wq")          # 3 MiB
            bq = pp.tile([128, 12], f32, tag="bq")
            wfc = pp.tile([128, 4, E], bf16, tag="wfc")        # 1 MiB
            bias = pp.tile([128, E], f32, tag="bias")          # 0.5 MiB
            qt = pp.tile([128, 4, L], bf16, tag="qt")          # Q^T 2 MiB
            kt = pp.tile([128, 4, L], bf16, tag="kt")          # K^T 2 MiB
            vt = sp.tile([128, 4, L], bf16, tag="vt")          # V^T staging 2 MiB
            # V natural layout, 80-elem stride; col 64 holds the ones column
            # so AV matmuls with lhsT [V|1] (M=65) produce rowsums for free
            v = pp.tile([128, H8, 16, 80], bf16, tag="v")      # 2.5 MiB
            onT = pp.tile([128, 4, L], bf16, tag="onT")        # attn out^T 2 MiB
            # ones row on partition 64 for the PE-side recip broadcast
            ones_t = pp.tile([128, 64], bf16, tag="ones_t")

            # ---- input DMAs (plain copies, split across both HWDGE queues)
            for e in range(8):
                nc.scalar.dma_start(
                    wq[:, e, :], w_qkv[e * 128 : (e + 1) * 128, :]
                )
                nc.sync.dma_start(xT[:, e, :], x[e * 128 : (e + 1) * 128, :])
                if e == 0:
                    nc.scalar.dma_start(bq[:], b_qkv[:])
            nc.sync.dma_start(wfc[:], w_fc.rearrange("(c p) e -> p c e", p=128))
            bfc_row = pp.tile([1, E], f32, tag="bfc_row")
            nc.sync.dma_start(bfc_row[:], b_fc[:])
            nc.gpsimd.partition_broadcast(bias[:], bfc_row[:])
            nc.vector.memset(v[:, :, :, 64:65], 1.0)
            nc.vector.memset(ones_t[:], 1.0)

            # exp table preload: the ~2.7us PSEUDO_LOAD_ACT_FUNC_SET runs
            # under the input DMAs instead of at the first real exp
            dummy = pp.tile([1, 8], f32, tag="dummy")
            nc.vector.memset(dummy[:], 0.0)
            nc.scalar.activation(dummy[:], dummy[:], Exp)

            # one DRAM tile per RS chunk: a sliced shared tile would give the
            # collectives tile-coarse false deps against later fc writes,
            # which stalls the sync queue (and transitively the DVE chain).
            # 8 small chunks keep each collective's gpsimd-queue occupancy
            # under the attention block period, so a pending normalization
            # broadcast never waits behind a running collective.
            rs_in = [
                dram.tile([256, E], bf16, name=f"rs_in{i}", tag=f"rs_in{i}")
                for i in range(8)
            ]
            PAIRS = [[0, 1], [2, 3], [4, 5], [6, 7]]
            rs_outc = [
                dram.tile([128, E], bf16, name=f"rs_outc{i}", tag=f"rs_outc{i}")
                for i in range(8)
            ]

            def rs_chunk(i):
                # ReduceScatter of 256 fc-output tokens (chunk i of 8):
                # core g of a pair gets tokens i*256 + g*128 + [0:128]
                nc.gpsimd.collective_compute(
                    "ReduceScatter", ADD, replica_groups=PAIRS,
                    ins=[rs_in[i].opt()], outs=[rs_outc[i].opt()],
                )

            with (
                tc.tile_pool(name="pst", bufs=2, space="PSUM") as pst_pool,
                tc.tile_pool(name="psav", bufs=1, space="PSUM") as psav_pool,
                tc.tile_pool(name="psbg", bufs=1, space="PSUM") as psbg_pool,
            ):
                # ---- HAM warm-up: ~40 junk matmuls into the (not yet
                # used) av slot while the input DMAs land, so the PE clock
                # gate is already at full rate when real work starts.
                warm = psav_pool.tile([128, LHALF], f32, tag="av", name="av")
                for _ in range(40):
                    nc.tensor.matmul(
                        warm[0:64, 0:512], wq[:, 0, 0:64], wq[:, 0, 0:512],
                        start=True, stop=True,
                    )

                # ---- background PE work: qkv projection groups and fc
                # chunks, emitted a few matmuls at a time from inside the
                # attention loop (fills PE slack, keeps the HAM clock warm).
                # Each unit is a closure emitting ~1 FD-512 matmul.
                bg_units = []

                def qkv_group(ft, tb):
                    """ft-tile [128f, 512tok]: 8 matmuls + bias eviction
                    into qt/kt/vt. bg tile is [128,512] double-buffered so
                    the eviction overlaps the next group's matmuls."""
                    bgt = psbg_pool.tile([128, 512], f32, tag="bg", name="bg")
                    for kc in range(8):
                        lhsT = wq[:, kc, ft * 128 : (ft + 1) * 128]
                        yield lambda kc=kc, lhsT=lhsT, bgt=bgt: (
                            nc.tensor.matmul(
                                bgt[:],
                                lhsT,
                                xT[:, kc, tb * 512 : (tb + 1) * 512],
                                start=(kc == 0),
                                stop=(kc == 7),
                            )
                        )
                    if ft < 4:
                        dst = qt[:, ft, tb * 512 : (tb + 1) * 512]
                    elif ft < 8:
                        dst = kt[:, ft - 4, tb * 512 : (tb + 1) * 512]
                    else:
                        dst = vt[:, ft - 8, tb * 512 : (tb + 1) * 512]

                    def evict(bgt=bgt, dst=dst, ft=ft, tb=tb):
                        nc.vector.tensor_scalar_add(dst, bgt[:], bq[:, ft : ft + 1])
                        if ft >= 8 and tb == 3:
                            # V^T -> V (token-major) via xbar transpose
                            for h in (2 * (ft - 8), 2 * (ft - 8) + 1):
                                nc.sync.dma_start_transpose(
                                    v[:, h, :, 0:DK],
                                    vt[(h % 2) * 64 : (h % 2) * 64 + 64, h // 2, :],
                                )

                    yield evict

                def fc_chunk(qb, t8):
                    """fc for one 128-token chunk: two 4-matmul column
                    halves on the [128,512] bg tile + bias/cast/DMA."""
                    t = qb * 8 + t8
                    ys = yp_pool.tile([128, E], bf16, tag="ys")
                    for e2 in range(2):
                        yp = psbg_pool.tile([128, 512], f32, tag="bg", name="bg")
                        for c in range(4):
                            lhsT = onT[:, c, t * 128 : (t + 1) * 128]
                            yield lambda c=c, e2=e2, lhsT=lhsT, yp=yp: (
                                nc.tensor.matmul(
                                    yp[:],
                                    lhsT,
                                    wfc[:, c, e2 * 512 : (e2 + 1) * 512],
                                    start=(c == 0),
                                    stop=(c == 3),
                                )
                            )

                        def evict_half(yp=yp, e2=e2):
                            nc.vector.tensor_tensor(
                                ys[:, e2 * 512 : (e2 + 1) * 512], yp[:],
                                bias[:, e2 * 512 : (e2 + 1) * 512], op=ADD,
                            )

                        yield evict_half

                    def send(ys=ys, qb=qb, t8=t8):
                        nc.sync.dma_start(
                            rs_in[4 * qb + t8 // 2][(t8 % 2) * 128 : (t8 % 2) * 128 + 128, :],
                            ys[:],
                        )

                    yield send

                def bg_push(gen):
                    # keep generators lazy: a group's PSUM tile is allocated
                    # when its first unit is pulled, so slot-cycling order
                    # matches emission order
                    bg_units.append(gen)

                def bg_run(n):
                    done = 0
                    while done < n and bg_units:
                        u = next(bg_units[0], None)
                        if u is None:
                            bg_units.pop(0)
                            continue
                        u()
                        done += 1

                def bg_run_all():
                    while bg_units:
                        u = next(bg_units[0], None)
                        if u is None:
                            bg_units.pop(0)
                            continue
                        u()

                # qkv emission order, per head-pair: V (all tokens, feeds
                # the transposes), Q for the qb=0 half only, K (all
                # tokens). Q's qb=1 halves are deferred to the end -- the
                # qb=1 blocks only need them much later.
                QKV_GROUPS = []
                for jp in range(4):
                    QKV_GROUPS += [(8 + jp, tb) for tb in range(4)]
                    QKV_GROUPS += [(jp, 0), (jp, 1)]
                    QKV_GROUPS += [(4 + jp, tb) for tb in range(4)]
                QKV_GROUPS += [(jp, tb) for jp in range(4) for tb in (2, 3)]
                for ft, tb in QKV_GROUPS:
                    bg_push(qkv_group(ft, tb))

                # ---- pre-attention: exactly the groups blocks 0-1 read
                # (V ft8 all tb, Q ft0 tb0-1, K ft4 all tb = 10 groups).
                # Consumers must not be emitted before their producers --
                # the tile dep tracker only orders backward in emission.
                bg_run(10 * 9)

                def attn_block(qb, h, bg_per_kk=2):
                    """One head, one q-half (1024 tokens): 16-kk pipelined
                    S -> exp -> AV with st double-buffered."""
                    j, p0 = h // 2, 64 * (h % 2)
                    av = psav_pool.tile([128, LHALF], f32, tag="av", name="av")
                    pts = {}

                    def emit_st(kk):
                        st = pst_pool.tile([128, LHALF], f32, tag="st", name="st")
                        for u in range(2):
                            q0 = qb * LHALF + u * 512
                            nc.tensor.matmul(
                                st[:, u * 512 : (u + 1) * 512],
                                kt[p0 : p0 + 64, j, kk * 128 : (kk + 1) * 128],
                                qt[p0 : p0 + 64, j, q0 : q0 + 512],
                                start=True,
                                stop=True,
                            )
                        pt = wp.tile([128, LHALF], bf16, tag="pt")
                        nc.scalar.activation(pt[:], st[:], Exp, scale=0.125)
                        pts[kk] = pt

                    def emit_av(kk):
                        pt = pts.pop(kk)
                        first, last = kk == 0, kk == 15
                        for u in range(2):
                            sl = slice(u * 512, (u + 1) * 512)
                            nc.tensor.matmul(
                                av[0:65, sl], v[:, h, kk, 0:65], pt[:, sl],
                                start=first, stop=last,
                            )

                    for kk in range(16):
                        emit_st(kk)
                        bg_run(bg_per_kk)
                        if kk > 0:
                            emit_av(kk - 1)
                    emit_av(15)

                    # early evict (frees av psum): unnormalized out^T.
                    # odd heads' rows must land on partitions 64:128 -> DMA
                    # shift through a bf16 staging tile. Deferred
                    # normalization via gpsimd partition_broadcast (overlaps
                    # the next block).
                    qsl = slice(qb * LHALF, (qb + 1) * LHALF)
                    srs = srs_pool.tile([128, 2 * LHALF], f32, tag="srs")
                    if p0 == 0:
                        nc.vector.tensor_copy(onT[0:64, j, qsl], av[0:64, :])
                    else:
                        tmp = wp.tile([64, LHALF], bf16, tag="tmp")
                        nc.vector.tensor_copy(tmp[:], av[0:64, :])
                        nc.sync.dma_start(onT[64:128, j, qsl], tmp[:])
                    nc.vector.tensor_copy(srs[64:65, 0:LHALF], av[64:65, :])
                    nc.sync.dma_start(srs[0:1, 0:LHALF], srs[64:65, 0:LHALF])
                    nc.gpsimd.partition_broadcast(
                        srs[:, LHALF : 2 * LHALF], srs[0:1, 0:LHALF]
                    )
                    nc.vector.reciprocal_approx_fast(
                        srs[:, LHALF : 2 * LHALF], srs[:, LHALF : 2 * LHALF]
                    )
                    nc.vector.tensor_tensor(
                        onT[p0 : p0 + 64, j, qsl], onT[p0 : p0 + 64, j, qsl],
                        srs[p0 : p0 + 64, LHALF : 2 * LHALF], op=MUL,
                    )

                # qb=0 half: 8 one-head blocks; remaining qkv groups drain
                # into their PE slack (front-loaded so ft deadlines for
                # later heads are met without stalls).
                for h, rate in zip(range(8), (4, 4, 4, 3, 3, 3, 3, 3)):
                    attn_block(0, h, rate)
                bg_run_all()
                # fc for qb=0 spreads over the early qb=1 blocks; each RS
                # chunk fires at a block boundary as soon as its 2 fc
                # chunks are in DRAM, so the collectives overlap attention
                # without ever making a broadcast queue behind them.
                for t8 in range(8):
                    bg_push(fc_chunk(0, t8))
                attn_block(1, 0, 2)
                rs_chunk(0)
                attn_block(1, 1, 2)
                rs_chunk(1)
                attn_block(1, 2, 2)
                rs_chunk(2)
                attn_block(1, 3, 2)
                bg_run_all()
                rs_chunk(3)
                for h in range(4, 8):
                    attn_block(1, h, 1)
                # fc for qb=1: the serial tail, pipelined 2 chunks per RS
                for i in range(4):
                    for t8 in (2 * i, 2 * i + 1):
                        bg_push(fc_chunk(1, t8))
                    bg_run_all()
                    rs_chunk(4 + i)
                # output copies last: earlier RS chunks are long done, so
                # only the last copy waits on anything
                for i in range(8):
                    nc.gpsimd.dma_start(
                        out[i * 128 : (i + 1) * 128, :], rs_outc[i][:]
                    )

    nc.finalize()
    return nc


def _prep_inputs(X, W_qkv, b_qkv, W_fc, b_fc):
    """Host-side shard + permute + cast. Returns in_maps for 8 cores."""
    X = np.asarray(X, dtype=np.float32)
    W_qkv = np.asarray(W_qkv, dtype=np.float32)
    b_qkv = np.asarray(b_qkv, dtype=np.float32)
    W_fc = np.asarray(W_fc, dtype=np.float32)
    b_fc = np.asarray(b_fc, dtype=np.float32)

    in_maps = []
    bfc_half = (0.5 * b_fc).astype(np.float32).reshape(1, E)
    for c in range(NCORES):
        b, g = divmod(c, 2)
        heads = np.arange(g * H8, (g + 1) * H8)
        # column order: all Q feats (head-major), then K, then V
        cols = np.concatenate(
            [
                np.concatenate([h * 3 * DK + off + np.arange(DK) for h in heads])
                for off in (0, DK, 2 * DK)
            ]
        )
        wq_sh = W_qkv[:, cols].astype(ml_dtypes.bfloat16)
        bq_sh = b_qkv[cols].astype(np.float32).reshape(12, 128).T.copy()
        wfc_sh = W_fc[g * FO : (g + 1) * FO, :].astype(ml_dtypes.bfloat16)
        in_maps.append(
            {
                "x": np.ascontiguousarray(X[b].T).astype(ml_dtypes.bfloat16),
                "w_qkv": wq_sh,
                "b_qkv": np.ascontiguousarray(bq_sh),
                "w_fc": wfc_sh,
                "b_fc": bfc_half,
            }
        )
    return in_maps


def run_kernel(inputs, trace=False):
    if "nc" not in _CACHE:
        _CACHE["nc"] = build_nc()
    nc = _CACHE["nc"]
    in_maps = _prep_inputs(**inputs)
    res = bass_utils.run_bass_kernel_spmd(
        nc, in_maps, core_ids=list(range(NCORES)), trace=trace
    )
    Y = np.empty((B, L, E), dtype=np.float32)
    for c in range(NCORES):
        b, g = divmod(c, 2)
        o = np.asarray(res.results[c]["out"], dtype=np.float32)
        # RS chunk i scattered tokens [i*256:(i+1)*256]; core g of a pair
        # holds tokens i*256 + g*128 + [0:128] in out rows [i*128:(i+1)*128]
        for i in range(8):
            Y[b, i * 256 + g * 128 : i * 256 + (g + 1) * 128, :] = (
                o[i * 128 : (i + 1) * 128]
            )
    return Y, res


def kernel(X, W_qkv, b_qkv, W_fc, b_fc):
    Y, _ = run_kernel(
        dict(X=X, W_qkv=W_qkv, b_qkv=b_qkv, W_fc=W_fc, b_fc=b_fc), trace=False
    )
    return Y
